# revision 1
# baseline (speedup 1.0000x reference)
"""Trainium2 Bass kernel for nn_MultiAgentsSummarizer (pointer-generator style
multi-agent summarizer distribution).

Math (per batch b, with T=64 target positions, A=4 agents, S=512 source tokens,
V=32000 vocab, EXT_V=33000 extended vocab):

    coef[t]   = sum_a agent_attn[t,a] * gen[t,a]
    out[t,v]  = coef[t] * vocab_probs[t,v]            (v <  V;  0 for v >= V)
    out[t, article[a,s]] += agent_attn[t,a]*(1-gen[t,a]) * agentwise_attn[t,a,s]

Strategy: one batch element per NeuronCore (B=8 = n_cores). Device work runs in
a v-major ("transposed") staging layout out[v, t]: each scatter destination is
one contiguous 256-byte DRAM row, served by GPSIMD dma_scatter_add (CCE add).
The host only reorders/relabels data (transposes, column permutation, index
tables) -- all floating-point arithmetic runs on device.

Staging is out_main [32768, 64] (v < 32768; 32768 = 256*128, and 32000 = 125*256
so the scaled-vocab base stream is exactly partitions 0..124 with no ragged
edge) plus out_hi [256, 64] (v - 32768). Rows >= 32000 have zero base and rely
on the PJRT-donated pre-zeroed output buffers.

dma_scatter_add RMW races on duplicate rows inside one call, so each call gets
one payload slot per unique destination row. Contribution layout (128-slot
chunks; payload of slot k lives at items[k % 128, (k//128)*T:...]):
  chunks 0..15  A-call singles, per-agent static ranges (tensor_scalar by c4)
  chunks 16,17  A-call merge slots M (duplicate groups, mixed agents)
  chunk  18     H-call (v >= 32768), mixed agents
  chunks 19,20  S1: rank-1 partners of M groups (same partition as the group)
  chunk  21     S2: rank-2 partners (partition-aligned with M chunk 16)
  chunk  22     S3: rank-1 partners of H rows (partition-aligned with H)
  chunk  23     S4: rank-3 partners (partition-aligned with M chunk 16)
Mixed-agent chunks (16..23) get their c4[t,a(slot)] factor from a tiny PE
matmul (c4T [4,64] x one-hot [4,1024]) instead of static ranges. Before the
scatter, duplicates are folded with four DVE adds (M += S1, M0 += S2, M0 += S4,
H += S3), leaving unique rows only. Unused slots carry zero payload and point
at host-chosen dump rows that the call never really targets (+0.0 RMW is
harmless). The program is fully static across cores; only tensor data varies.
"""

import numpy as np

import concourse.bacc as bacc
import concourse.bass as bass
import concourse.mybir as mybir
import concourse.tile as tile
from concourse.bass_utils import run_bass_kernel_spmd

B, T, A, S = 8, 64, 4, 512
V, EXT_V = 32000, 33000
P = 128
KC = A * S  # 2048 contributions per batch element

MAIN = 32768  # out_main rows; = 256*128
HI = 256  # out_hi rows; idx = v - MAIN
SPP = MAIN // P  # 256 rows per partition stripe
VPART = V // SPP  # 125 partitions carry vocab rows
ROW_CHUNKS = [43, 43, 43, 43, 42, 42]  # c-chunks of the base stream (sum = SPP)

N_SINGLE_CH = 16  # chunks 0..15: per-agent singles (512 per agent)
M_CH = (16, 17)  # A-call merge chunks
H_CH = 18
S1_CH = (19, 20)
S2_CH = 21
S3_CH = 22
S4_CH = 23
NCH = 24
NSLOT = NCH * P  # 3072
A_CAP = 18 * P  # 2304 (chunks 0..17)
H_CAP = P
SPECIAL0 = 16 * P  # first matmul-multiplied slot

_prog = None


class _nullctx:
    def __enter__(self):
        return None

    def __exit__(self, *a):
        return False


def _build_program(loop_n=None, ablate=()):
    """loop_n: on-device repeat loop (bench variant; outputs then meaningless).
    ablate: subset of {"scatter", "base", "items", "fence", "prep"} (bench)."""
    ablate = set(ablate)
    nc = bacc.Bacc("TRN2", target_bir_lowering=False)
    f32 = mybir.dt.float32
    vocab_t = nc.dram_tensor("vocab_t", [V, T], f32, kind="ExternalInput")
    agat_t = nc.dram_tensor("agat_t", [A, T], f32, kind="ExternalInput")
    gen_t = nc.dram_tensor("gen_t", [A, T], f32, kind="ExternalInput")
    attn_slots = nc.dram_tensor("attn_slots", [T, NSLOT], f32, kind="ExternalInput")
    onehot_t = nc.dram_tensor("onehot_t", [A, NSLOT - SPECIAL0], f32, kind="ExternalInput")
    ident_in = nc.dram_tensor("ident_in", [T, T], f32, kind="ExternalInput")
    idx_a = nc.dram_tensor("idx_a", [P, A_CAP // 16], mybir.dt.int16, kind="ExternalInput")
    idx_h = nc.dram_tensor("idx_h", [P, H_CAP // 16], mybir.dt.int16, kind="ExternalInput")
    out_main = nc.dram_tensor("out_main", [MAIN, T], f32, kind="ExternalOutput")
    out_hi = nc.dram_tensor("out_hi", [HI, T], f32, kind="ExternalOutput")

    with tile.TileContext(nc) as tc:
        with (
            tc.tile_pool(name="small", bufs=1) as small,
            tc.tile_pool(name="wpool", bufs=1) as wpool,
            tc.tile_pool(name="vt", bufs=3) as vtp,
            tc.tile_pool(name="sc", bufs=3) as scp,
            tc.tile_pool(name="psum1", bufs=1, space="PSUM") as psum1,
            tc.tile_pool(name="psumc", bufs=2, space="PSUM") as psumc,
            tc.tile_pool(name="psum", bufs=4, space="PSUM") as psum,
            (tc.For_i(0, loop_n, 1) if loop_n else _nullctx()),
        ):
            # ---- per-(t,a) coefficients ----
            agat_sb = small.tile([A, T], f32)
            gen_sb = small.tile([A, T], f32)
            nc.sync.dma_start(agat_sb[:], agat_t[:])
            nc.sync.dma_start(gen_sb[:], gen_t[:])

            prod = small.tile([A, T], f32)
            nc.vector.tensor_mul(prod[:], agat_sb[:], gen_sb[:])
            ones4 = small.tile([A, P], f32)
            nc.vector.memset(ones4[:], 1.0)
            coef_ps = psum1.tile([P, T], f32, space="PSUM")
            nc.tensor.matmul(coef_ps[:], lhsT=ones4[:], rhs=prod[:], start=True, stop=True)
            coef_bc = small.tile([P, T], f32)  # coef[t] on all partitions
            nc.vector.tensor_copy(coef_bc[:], coef_ps[:])

            one4 = small.tile([A, T], f32)
            nc.vector.memset(one4[:], 1.0)
            c4t = small.tile([A, T], f32)  # c4T[a, t] = agent_attn*(1-gen)
            nc.vector.tensor_sub(c4t[:], one4[:], gen_sb[:])
            nc.vector.tensor_mul(c4t[:], c4t[:], agat_sb[:])

            # ---- small loads issued early (ahead of vocab in the qSP FIFO) ----
            attn_sb = wpool.tile([T, NSLOT], f32)
            nc.sync.dma_start(attn_sb[:], attn_slots[:])
            onehot_sb = small.tile([A, NSLOT - SPECIAL0], f32)
            nc.sync.dma_start(onehot_sb[:], onehot_t[:])
            identT = small.tile([T, T], f32)
            nc.sync.dma_start(identT[:], ident_in[:])
            ia = small.tile([P, A_CAP // 16], mybir.dt.int16)
            nc.sync.dma_start(ia[:], idx_a[:])
            ih = small.tile([P, H_CAP // 16], mybir.dt.int16)
            nc.sync.dma_start(ih[:], idx_h[:])

            # c4 [64, 4] (for tensor_scalar per-partition use)
            c4_ps = psum1.tile([T, A], f32, space="PSUM")
            nc.tensor.transpose(c4_ps[:], c4t[:], identT[:A, :A])
            c4 = small.tile([T, A], f32)
            nc.vector.tensor_copy(c4[:], c4_ps[:])

            # ---- base: out_main[v,t] = coef[t] * vocab_t[v,t] on 125 stripes ----
            if "base" not in ablate:
                vview = vocab_t[:].rearrange("(p c) t -> p c t", p=VPART, c=SPP)
                oview = out_main[0 : VPART * SPP, :].rearrange(
                    "(p c) t -> p c t", p=VPART, c=SPP
                )
                r0 = 0
                for rj in ROW_CHUNKS:
                    vt = vtp.tile([VPART, rj * T], f32, tag="vt")
                    nc.sync.dma_start(vt[:], vview[:, r0 : r0 + rj, :])
                    sc = scp.tile([VPART, rj * T], f32, tag="sc")
                    nc.vector.tensor_tensor(
                        out=sc[:].rearrange("p (c t) -> p c t", c=rj),
                        in0=vt[:].rearrange("p (c t) -> p c t", c=rj),
                        in1=coef_bc[:VPART, None, :].to_broadcast([VPART, rj, T]),
                        op=mybir.AluOpType.mult,
                    )
                    nc.scalar.dma_start(oview[:, r0 : r0 + rj, :], sc[:])
                    r0 += rj

            # ---- scatter payload w[t, slot] ----
            do_items = "items" not in ablate
            w = wpool.tile([T, NSLOT], f32)
            if do_items:
                for a in range(A):  # singles: chunks 0..15, static per-a ranges
                    nc.vector.tensor_scalar(
                        out=w[:, a * 512 : (a + 1) * 512],
                        in0=attn_sb[:, a * 512 : (a + 1) * 512],
                        scalar1=c4[:, a : a + 1],
                        scalar2=None,
                        op0=mybir.AluOpType.mult,
                    )
                # special chunks 16..23: cmul = c4T.T @ onehot via PE
                nspec = NSLOT - SPECIAL0  # 1024
                for j0 in range(0, nspec, 512):
                    j1 = min(j0 + 512, nspec)
                    cm = psumc.tile([T, j1 - j0], f32, space="PSUM", tag="cmul")
                    nc.tensor.matmul(
                        cm[:], lhsT=c4t[:], rhs=onehot_sb[:, j0:j1], start=True, stop=True
                    )
                    nc.vector.tensor_tensor(
                        out=w[:, SPECIAL0 + j0 : SPECIAL0 + j1],
                        in0=attn_sb[:, SPECIAL0 + j0 : SPECIAL0 + j1],
                        in1=cm[:],
                        op=mybir.AluOpType.mult,
                    )
                items = wpool.tile([P, NCH * T], f32)
                for ch in range(NCH):
                    tp = psum.tile([P, T], f32, space="PSUM")
                    nc.tensor.transpose(tp[:], w[:, ch * P : (ch + 1) * P], identT[:])
                    nc.vector.tensor_copy(items[:, ch * T : (ch + 1) * T], tp[:])

                # fold duplicates: M += S1, M0 += S2, M0 += S4, H += S3
                def _add(dst_ch, src_ch, n=1):
                    nc.vector.tensor_add(
                        out=items[:, dst_ch * T : (dst_ch + n) * T],
                        in0=items[:, dst_ch * T : (dst_ch + n) * T],
                        in1=items[:, src_ch * T : (src_ch + n) * T],
                    )

                _add(M_CH[0], S1_CH[0], n=2)
                _add(M_CH[0], S2_CH)
                _add(M_CH[0], S4_CH)
                _add(H_CH, S3_CH)

            # ---- scatter-add calls ----
            if "scatter" not in ablate:
                prep = "prep" in ablate
                kw = {}
                sems = []
                if prep:
                    sem_a = nc.alloc_semaphore("scat_a")
                    sem_h = nc.alloc_semaphore("scat_h")
                nc.gpsimd.dma_scatter_add(
                    out_main[:, :],
                    items[:, 0 : 18 * T].rearrange("p (c t) -> p c t", c=18),
                    ia[:],
                    A_CAP,
                    A_CAP,
                    T,
                    **({"prepare_only": True, "sem": sem_a} if prep else {}),
                )
                nc.gpsimd.dma_scatter_add(
                    out_hi[:, :],
                    items[:, H_CH * T : (H_CH + 1) * T].rearrange("p (c t) -> p c t", c=1),
                    ih[:],
                    H_CAP,
                    H_CAP,
                    T,
                    **({"prepare_only": True, "sem": sem_h} if prep else {}),
                )
                if prep:
                    nc.gpsimd.trigger_dma(count=None)

            if "fence" in ablate:
                for h, o in enumerate([out_main, out_hi]):
                    fr = small.tile([P, T], f32, tag=f"fence{h}")
                    nc.sync.dma_start(fr[:], o[0:P, :])
                    fs = small.tile([P, 1], f32, tag=f"fsum{h}")
                    nc.vector.reduce_sum(out=fs[:], in_=fr[:], axis=mybir.AxisListType.X)

    nc.compile()
    return nc


def _pack_core(vocab_b, gen_b, agat_b, attn_b, article_b):
    """Host-side data layout for one batch element (no float arithmetic)."""
    v = article_b.reshape(-1).astype(np.int64)  # contribution k = a*S + s
    a_of = np.repeat(np.arange(A), S)
    attn_flat = np.ascontiguousarray(attn_b.reshape(T, KC), dtype=np.float32)

    slots = np.zeros((T, NSLOT), np.float32)
    onehot = np.zeros((A, NSLOT - SPECIAL0), np.float32)
    tab_a = np.full(A_CAP, -1, np.int64)
    tab_h = np.full(H_CAP, -1, np.int64)

    # group contributions by destination row
    groups = {}
    for k in range(KC):
        groups.setdefault(int(v[k]), []).append(k)

    def put(ch, p, k):  # place contribution k at payload slot (partition p, chunk ch)
        slot = ch * P + p
        slots[:, slot] = attn_flat[:, k]
        if slot >= SPECIAL0:
            onehot[int(a_of[k]), slot - SPECIAL0] = 1.0

    a_cnt = np.zeros(A, np.int64)
    m_cnt = 0  # merge groups placed (over chunks 16,17 / S1 19,20)
    h_cnt = 0
    used_h = set()
    for vv, ks in groups.items():
        if vv >= MAIN:
            if len(ks) > 2:
                raise RuntimeError("hi row multiplicity > 2 unsupported")
            p = h_cnt
            h_cnt += 1
            if h_cnt > P:
                raise RuntimeError("hi capacity exceeded")
            tab_h[p] = vv - MAIN
            used_h.add(vv - MAIN)
            put(H_CH, p, ks[0])
            if len(ks) > 1:
                put(S3_CH, p, ks[1])
        elif len(ks) == 1:
            aa = int(a_of[ks[0]])
            if a_cnt[aa] >= 512:
                raise RuntimeError("singles capacity exceeded")
            pos = aa * 512 + a_cnt[aa]  # slot among chunks 0..15
            a_cnt[aa] += 1
            slots[:, pos] = attn_flat[:, ks[0]]
            tab_a[pos] = vv
        else:
            if len(ks) > 4:
                raise RuntimeError("row multiplicity > 4 unsupported")
            if len(ks) > 2:  # needs S2/S4 -> must sit in M chunk 16
                if m_cnt >= P:
                    raise RuntimeError("deep-duplicate capacity exceeded")
                p, ch_i = m_cnt, 0
            else:
                if m_cnt >= 2 * P:
                    raise RuntimeError("duplicate capacity exceeded")
                p, ch_i = m_cnt % P, m_cnt // P
            m_cnt += 1
            put(M_CH[ch_i], p, ks[0])
            put(S1_CH[ch_i], p, ks[1])
            if len(ks) > 2:
                put(S2_CH, p, ks[2])
            if len(ks) > 3:
                put(S4_CH, p, ks[3])
            tab_a[(16 + ch_i) * P + p] = vv

    # dump rows for unused A slots: rows this call never really targets
    used_a = set(tab_a[tab_a >= 0].tolist())
    free = 0
    for pos in range(A_CAP):
        if tab_a[pos] < 0:
            while free in used_a:
                free += 1
            tab_a[pos] = free
            free += 1
    free = 0
    for pos in range(H_CAP):
        if tab_h[pos] < 0:
            while free in used_h:
                free += 1
            tab_h[pos] = free
            free += 1

    def rep16(tab):  # entry k at [k%16, k//16], replicated to 128 partitions
        return np.ascontiguousarray(np.tile(tab.astype(np.int16).reshape(-1, 16).T, (8, 1)))

    return {
        "vocab_t": np.ascontiguousarray(vocab_b.T, dtype=np.float32),
        "agat_t": np.ascontiguousarray(agat_b.T, dtype=np.float32),
        "gen_t": np.ascontiguousarray(gen_b.T, dtype=np.float32),
        "attn_slots": slots,
        "onehot_t": onehot,
        "ident_in": np.eye(T, dtype=np.float32),
        "idx_a": rep16(tab_a),
        "idx_h": rep16(tab_h),
    }


def kernel(vocab_probs, generation_probs, agentwise_attn, agent_attn, article):
    global _prog
    vocab_probs = np.asarray(vocab_probs, dtype=np.float32)
    generation_probs = np.asarray(generation_probs, dtype=np.float32)
    agentwise_attn = np.asarray(agentwise_attn, dtype=np.float32)
    agent_attn = np.asarray(agent_attn, dtype=np.float32)
    article = np.asarray(article)

    if _prog is None:
        _prog = _build_program()

    in_maps = [
        _pack_core(
            vocab_probs[b], generation_probs[b], agat_b=agent_attn[b],
            attn_b=agentwise_attn[b], article_b=article[b],
        )
        for b in range(B)
    ]
    res = run_bass_kernel_spmd(_prog, in_maps, core_ids=list(range(B)))
    full = np.empty((B, T, EXT_V), np.float32)
    for b, r in enumerate(res.results):
        full[b, :, :MAIN] = r["out_main"].T
        full[b, :, MAIN:] = r["out_hi"][: EXT_V - MAIN].T
    return full



# revision 4
# speedup vs baseline: 21.4555x; 21.4555x over previous
"""Trainium2 Bass kernel for nn_MultiAgentsSummarizer (pointer-generator style
multi-agent summarizer distribution).

Math (per batch b, with T=64 target positions, A=4 agents, S=512 source tokens,
V=32000 vocab, EXT_V=33000 extended vocab):

    coef[t]   = sum_a agent_attn[t,a] * gen[t,a]
    out[t,v]  = coef[t] * vocab_probs[t,v]            (v <  V;  0 for v >= V)
    out[t, article[a,s]] += agent_attn[t,a]*(1-gen[t,a]) * agentwise_attn[t,a,s]

Strategy: one batch element per NeuronCore (B=8 = n_cores). Device work runs
v-major in 258 blocks of 128 rows (NB*128 = 33024 >= EXT_V). Both the base
term and the scatter term for a block are PE matmuls accumulating into the
same PSUM region:

  base    psum[p,t] += sum_k vocabT[k=t, blk*128+p] * coefdiag[k=t, t']
          (lhsT = vocab slice in natural [T, V] layout, rhs = diag(coef);
          vocab halves stacked on partitions so base partition is 0 or 64)
  scatter psum[p,t] += sum_k sel[k, p] * w[k, t]
          (host sorts the A*S=2048 contributions by destination block;
          32 slots per block, 3 blocks per 128-slot chunk at partition
          offsets {0,32,64}; sel is the 0/1 row-selector; duplicates need
          no special handling -- the matmul sums them)

w[k,t] = agentwise_attn[t,a(k),s(k)] * c4[t,a(k)] is built on device: a tiny
one-hot matmul gathers c4 rows per slot (c4sel), then one DVE multiply per
4-chunk span. PSUM group rule (one pending accumulation group per 2KB bank)
is satisfied by emitting base->scatter back-to-back per block. Groups of 6
blocks share one PSUM bank [128, 384]; ACT/DVE alternate copying PSUM to
bf16 tiles which stream out with per-partition-contiguous DMA. All heavy
tensors travel bf16 (tolerance 2e-2; bf16 error ~3e-3). The host only
reorders/relabels/casts -- all arithmetic runs on device.
"""

import numpy as np
import ml_dtypes

import concourse.bacc as bacc
import concourse.bass as bass
import concourse.mybir as mybir
import concourse.tile as tile
from concourse.bass_utils import run_bass_kernel_spmd

B, T, A, S = 8, 64, 4, 512
V, EXT_V = 32000, 33000
KC = A * S  # 2048 contributions per batch element
BF = ml_dtypes.bfloat16

NB = 258  # v-blocks of 128 rows; NB*128 = 33024 >= EXT_V
VB = 250  # blocks with vocab rows (V = 250*128 exactly)
SPB = 32  # payload slots per block (max contributions per block)
BPC = 3  # blocks per 128-slot chunk (partition offsets 0/32/64)
NCHUNK = NB // BPC  # 86
GB = 6  # blocks per psum group (= 2 chunks); 258 = 43*6
NG = NB // GB  # 43
VHALF = V // 2  # 16000 columns per stacked vocab half

_prog = None


class _nullctx:
    def __enter__(self):
        return None

    def __exit__(self, *a):
        return False


def _build_program(loop_n=None, ablate=()):
    """loop_n: on-device repeat loop (bench variant; outputs then meaningless).
    ablate: subset of {"scatter", "base", "w"} (bench attribution)."""
    ablate = set(ablate)
    nc = bacc.Bacc("TRN2", target_bir_lowering=False)
    f32 = mybir.dt.float32
    b16 = mybir.dt.bfloat16
    vocab_in = nc.dram_tensor("vocab_in", [128, VHALF], b16, kind="ExternalInput")
    attn_in = nc.dram_tensor("attn_in", [128, NCHUNK * T], b16, kind="ExternalInput")
    sel_in = nc.dram_tensor("sel_in", [128, NCHUNK * 128], b16, kind="ExternalInput")
    oha_in = nc.dram_tensor("oha_in", [A, NCHUNK * 128], b16, kind="ExternalInput")
    gen_t = nc.dram_tensor("gen_t", [A, T], f32, kind="ExternalInput")
    agat_t = nc.dram_tensor("agat_t", [A, T], f32, kind="ExternalInput")
    mask2_in = nc.dram_tensor("mask2_in", [128, T], b16, kind="ExternalInput")
    out_img = nc.dram_tensor("out_img", [128, NB * T], b16, kind="ExternalOutput")

    with tile.TileContext(nc) as tc:
        with (
            tc.tile_pool(name="small", bufs=1) as small,
            tc.tile_pool(name="big", bufs=1) as big,
            tc.tile_pool(name="outp", bufs=4) as outp,
            tc.tile_pool(name="psc", bufs=1, space="PSUM") as psc,
            tc.tile_pool(name="psg", bufs=2, space="PSUM") as psg,
            tc.tile_pool(name="psa", bufs=4, space="PSUM") as psa,
            (tc.For_i(0, loop_n, 1) if loop_n else _nullctx()),
        ):
            # ---- tiny loads (SP queue) ----
            gen_sb = small.tile([A, T], f32)
            nc.sync.dma_start(gen_sb[:], gen_t[:])
            agat_sb = small.tile([A, T], f32)
            nc.sync.dma_start(agat_sb[:], agat_t[:])
            mask2 = small.tile([128, T], b16)
            nc.sync.dma_start(mask2[:], mask2_in[:])
            ohasb = small.tile([A, NCHUNK * 128], b16)
            nc.sync.dma_start(ohasb[:], oha_in[:])

            # ---- big loads (SP queue; per-partition contiguous) ----
            asb = big.tile([128, NCHUNK * T], b16)
            nc.sync.dma_start(asb[:], attn_in[:])
            selsb = big.tile([128, NCHUNK * 128], b16)
            nc.sync.dma_start(selsb[:], sel_in[:])
            vsb = big.tile([128, VHALF], b16)
            NVD = 8  # vocab load split for pipelining
            vw = VHALF // NVD
            for k in range(NVD):
                nc.sync.dma_start(
                    vsb[:, k * vw : (k + 1) * vw], vocab_in[:, k * vw : (k + 1) * vw]
                )

            # ---- coefficients ----
            prod = small.tile([A, T], f32)
            nc.vector.tensor_mul(prod[:], agat_sb[:], gen_sb[:])
            c4t_f = small.tile([A, T], f32)  # agent_attn*(1-gen) = agat - prod
            nc.vector.tensor_sub(c4t_f[:], agat_sb[:], prod[:])
            c4t = small.tile([A, T], b16)
            nc.vector.tensor_copy(c4t[:], c4t_f[:])

            ones4 = small.tile([A, 128], f32)
            nc.vector.memset(ones4[:], 1.0)
            coef_ps = psc.tile([128, T], f32, space="PSUM")
            nc.tensor.matmul(coef_ps[:], lhsT=ones4[:], rhs=prod[:], start=True, stop=True)
            coef_bc = small.tile([128, T], b16)
            nc.scalar.copy(coef_bc[:], coef_ps[:])
            coefdiag = small.tile([128, T], b16)  # rows 0..63 & 64..127 = diag(coef)
            nc.vector.tensor_mul(coefdiag[:], mask2[:], coef_bc[:])
            zlhs = small.tile([64, 128], b16)
            nc.vector.memset(zlhs[:], 0.0)

            # ---- payload w[slot, t] = attn[slot, t] * c4[t, a(slot)] ----
            wsb = big.tile([128, NCHUNK * T], b16)
            if "w" not in ablate:
                CPG = 4  # chunks per c4sel psum tile
                for cg in range((NCHUNK + CPG - 1) // CPG):
                    c0 = cg * CPG
                    c1 = min(c0 + CPG, NCHUNK)
                    ncr = c1 - c0
                    c4s = psg.tile([128, CPG * T], f32, space="PSUM", tag="c4s")
                    for j in range(ncr):
                        nc.tensor.matmul(
                            c4s[:, j * T : (j + 1) * T],
                            lhsT=ohasb[:, (c0 + j) * 128 : (c0 + j + 1) * 128],
                            rhs=c4t[:],
                            start=True,
                            stop=True,
                        )
                    nc.vector.tensor_mul(
                        wsb[:, c0 * T : c1 * T],
                        asb[:, c0 * T : c1 * T],
                        c4s[:, 0 : ncr * T],
                    )

            # ---- main loop: 43 groups of 6 blocks ----
            do_base = "base" not in ablate
            do_scat = "scatter" not in ablate
            for g in range(NG):
                acc = psa.tile([128, GB * T], f32, space="PSUM", tag="acc")
                for i in range(GB):
                    j = g * GB + i  # block index
                    reg = acc[:, i * T : (i + 1) * T]
                    started = False
                    if do_base and j < VB:
                        if j < VB // 2:
                            lhs = vsb[0:64, j * 128 : (j + 1) * 128]
                            rhs = coefdiag[0:64, :]
                        else:
                            lhs = vsb[64:128, (j - VB // 2) * 128 : (j - VB // 2 + 1) * 128]
                            rhs = coefdiag[64:128, :]
                        nc.tensor.matmul(reg, lhsT=lhs, rhs=rhs, start=True, stop=not do_scat)
                        started = True
                    elif do_scat:
                        # zero-start from tile position (0,0): HW rejects
                        # start=True matmuls at row-offset tile positions
                        nc.tensor.matmul(reg, lhsT=zlhs[:], rhs=coefdiag[0:64, :], start=True, stop=False)
                        started = True
                    if do_scat:
                        c = 2 * g + i // BPC
                        off = (i % BPC) * 32
                        nc.tensor.matmul(
                            reg,
                            lhsT=selsb[off : off + 32, c * 128 : (c + 1) * 128],
                            rhs=wsb[off : off + 32, c * T : (c + 1) * T],
                            start=not started,
                            stop=True,
                        )
                    if not started and not do_scat:
                        # ablation fallback: keep region defined
                        nc.tensor.matmul(
                            reg, lhsT=ones4[:], rhs=prod[:], start=True, stop=True
                        )
                out_t = outp.tile([128, GB * T], b16, tag="out")
                if g % 2 == 0:
                    nc.scalar.copy(out_t[:], acc[:])
                else:
                    nc.vector.tensor_copy(out_t[:], acc[:])
                nc.gpsimd.dma_start(out_img[:, g * GB * T : (g + 1) * GB * T], out_t[:])

    nc.compile()
    return nc


_MASK2 = (np.arange(128)[:, None] % 64 == np.arange(T)[None, :]).astype(BF)


def _pack_core(vocab_b, gen_b, agat_b, attn_b, article_b):
    """Host-side data layout for one batch element (reorder/relabel/cast only)."""
    vocab_img = np.ascontiguousarray(
        vocab_b.reshape(T, 2, VHALF).transpose(1, 0, 2).reshape(128, VHALF)
    ).astype(BF)

    v = article_b.reshape(-1).astype(np.int64)  # contribution k = a*S + s
    blk = v >> 7
    part = v & 127
    order = np.argsort(blk, kind="stable")
    blk_s = blk[order]
    part_s = part[order]
    counts = np.bincount(blk_s, minlength=NB)
    if counts.max() > SPB:
        raise RuntimeError(f"block capacity exceeded: {counts.max()} > {SPB}")
    starts = np.zeros(NB + 1, np.int64)
    np.cumsum(counts, out=starts[1:])
    rank = np.arange(KC) - starts[blk_s]
    srow = (blk_s // BPC) * 128 + (blk_s % BPC) * 32 + rank  # slot row

    attn_kt = attn_b.reshape(T, KC).T  # [k, t]
    Wf = np.zeros((NCHUNK * 128, T), np.float32)
    Wf[srow] = attn_kt[order]
    attn_img = np.ascontiguousarray(
        Wf.reshape(NCHUNK, 128, T).transpose(1, 0, 2).reshape(128, NCHUNK * T)
    ).astype(BF)

    Sf = np.zeros((NCHUNK * 128, 128), np.float32)
    Sf[srow, part_s] = 1.0
    sel_img = np.ascontiguousarray(
        Sf.reshape(NCHUNK, 128, 128).transpose(1, 0, 2).reshape(128, NCHUNK * 128)
    ).astype(BF)

    a_of = np.arange(KC) // S
    Oh = np.zeros((A, NCHUNK * 128), np.float32)
    Oh[a_of[order], srow] = 1.0

    return {
        "vocab_in": vocab_img,
        "attn_in": attn_img,
        "sel_in": sel_img,
        "oha_in": Oh.astype(BF),
        "gen_t": np.ascontiguousarray(gen_b.T, dtype=np.float32),
        "agat_t": np.ascontiguousarray(agat_b.T, dtype=np.float32),
        "mask2_in": _MASK2,
    }


def _unpack(res):
    """out_img [128, NB*T] bf16 -> [T, EXT_V] f32."""
    x = np.asarray(res["out_img"]).astype(np.float32).reshape(128, NB, T)
    return x.transpose(2, 1, 0).reshape(T, NB * 128)[:, :EXT_V]


def kernel(vocab_probs, generation_probs, agentwise_attn, agent_attn, article):
    global _prog
    vocab_probs = np.asarray(vocab_probs, dtype=np.float32)
    generation_probs = np.asarray(generation_probs, dtype=np.float32)
    agentwise_attn = np.asarray(agentwise_attn, dtype=np.float32)
    agent_attn = np.asarray(agent_attn, dtype=np.float32)
    article = np.asarray(article)

    if _prog is None:
        _prog = _build_program()

    in_maps = [
        _pack_core(
            vocab_probs[b], generation_probs[b], agat_b=agent_attn[b],
            attn_b=agentwise_attn[b], article_b=article[b],
        )
        for b in range(B)
    ]
    res = run_bass_kernel_spmd(_prog, in_maps, core_ids=list(range(B)))
    full = np.empty((B, T, EXT_V), np.float32)
    for b, r in enumerate(res.results):
        full[b] = _unpack(r)
    return full
